# revision 9
# baseline (speedup 1.0000x reference)
"""Trainium2 Bass kernel for nn_MetapathContextEncoder.

Algorithm (math-equivalent to the reference, with the softmax max-shift
dropped — valid because exp(t)/sum(exp(t)) is shift-invariant and t is
O(6) here, far from fp32 overflow):

    k = h_src @ Wk.T ; q = h_dst @ Wq.T (per-head split, dk=16)
    e[E,H]   = exp(scale * <k[src], q[dst]>_head)
    denom    = segsum_dst(e) + exp(scale * <dst_k, q>_head)
    out      = (segsum_dst(e * h_src[src]) + self_e * h_dst) / denom

Sharding: dst-node range sharding.  Core c owns dst rows
[c*6250, (c+1)*6250) and ALL edges pointing at them (host buckets edges
by dst via one argsort).  No collectives needed - each core's outputs
are disjoint.  Within a core, edges are grouped per 128-dst block; the
segment sums become one-hot matmuls accumulated in PSUM.
"""

import sys

for _p in ("/opt/trn_rl_repo",):
    if _p not in sys.path:
        sys.path.append(_p)

from contextlib import ExitStack

import ml_dtypes
import numpy as np

import concourse.bacc as bacc
import concourse.tile as tile
from concourse import mybir
from concourse.bass import IndirectOffsetOnAxis

P = 128
D = 128
H = 8
DK = 16
NCORES = 8
GROUP = 4          # 128-edge sub-chunks fused per gather / wide op
SCALE = 0.25       # 1/sqrt(dk)
PAD_DREL = 300.0   # dst_rel sentinel for padding slots (no one-hot match)

F32 = mybir.dt.float32
BF16 = mybir.dt.bfloat16
I32 = mybir.dt.int32


# --------------------------------------------------------------------------
# device program
# --------------------------------------------------------------------------

def build_program(n_src, nblk, S, mm_dt=F32, debug_dump=False):
    """Build + compile the per-core Bass program (identical on all cores).

    S = sub-chunks (of 128 edge slots) per dst block, multiple of GROUP.
    mm_dt: dtype view used for the big matmuls (F32 or float32r).
    """
    nc = bacc.Bacc(
        "TRN2",
        target_bir_lowering=False,
        debug=False,
        enable_asserts=False,
        num_devices=NCORES,
    )
    ndp = nblk * P  # padded dst rows per core

    hsrc = nc.dram_tensor("hsrc", [n_src, D], F32, kind="ExternalInput").ap()
    hdst = nc.dram_tensor("hdst", [ndp, D], F32, kind="ExternalInput").ap()
    wkT = nc.dram_tensor("wkT", [D, D], F32, kind="ExternalInput").ap()
    wqT = nc.dram_tensor("wqT", [D, D], F32, kind="ExternalInput").ap()
    srcs = nc.dram_tensor("srcs", [nblk, P, S], I32, kind="ExternalInput").ap()
    dcol = nc.dram_tensor("dcol", [nblk, P, S], F32, kind="ExternalInput").ap()
    drow = nc.dram_tensor("drow", [nblk, S * P], BF16, kind="ExternalInput").ap()
    iotar = nc.dram_tensor("iotar", [P, P], F32, kind="ExternalInput").ap()
    iotac = nc.dram_tensor("iotac", [P, 1], F32, kind="ExternalInput").ap()
    hmask = nc.dram_tensor("hmask", [P, H], F32, kind="ExternalInput").ap()
    ident = nc.dram_tensor("ident", [P, P], F32, kind="ExternalInput").ap()
    onesb = nc.dram_tensor("onesb", [1, P], BF16, kind="ExternalInput").ap()
    out = nc.dram_tensor("out", [ndp, D], F32, kind="ExternalOutput").ap()
    dbg = {}
    if debug_dump:
        for nm in ("g4", "p4", "pt4", "gt", "qesb", "prod", "cat",
                   "drps", "krps", "trow"):
            dbg[nm] = nc.dram_tensor(
                "dbg_" + nm, [P, 544], F32, kind="ExternalOutput").ap()

    GP = GROUP * P   # 512
    GH = GROUP * H   # 32
    GC = GROUP * (D + H)  # 544  (msg | e) rhs width per group

    def mmv(ap):
        return ap.bitcast(mm_dt) if mm_dt != F32 else ap

    with tile.TileContext(nc) as tc, ExitStack() as ctx:
        const = ctx.enter_context(tc.tile_pool(name="const", bufs=1))
        sb = ctx.enter_context(tc.tile_pool(name="sb", bufs=2))
        gp = ctx.enter_context(tc.tile_pool(name="gp", bufs=3))
        ps = ctx.enter_context(tc.tile_pool(name="ps", bufs=1, space="PSUM"))
        psa = ctx.enter_context(tc.tile_pool(name="psa", bufs=2, space="PSUM"))

        def cload(dram_ap, shape, dt, tag):
            t = const.tile(shape, dt, tag=tag)
            nc.sync.dma_start(t[:], dram_ap)
            return t

        iotart = cload(iotar[:, :], [P, P], F32, "iotar")
        iotact = cload(iotac[:, :], [P, 1], F32, "iotac")
        hmaskt = cload(hmask[:, :], [P, H], F32, "hmask")
        identt = cload(ident[:, :], [P, P], F32, "ident")
        onesbt = cload(onesb[:, :], [1, P], BF16, "onesb")
        wkTt = cload(wkT[:, :], [D, D], F32, "wkT")
        wqTt = cload(wqT[:, :], [D, D], F32, "wqT")

        # resident q table [d_local(128), nblk*128] - block b at cols [b*128,(b+1)*128)
        qsb = const.tile([P, nblk * P], F32, tag="qsb")

        # ---------------- prologue: q = h_dst @ Wq.T per block ----------------
        for b in range(nblk):
            hd = sb.tile([P, D], F32, tag="hd")
            nc.sync.dma_start(hd[:], hdst[b * P:(b + 1) * P, :])
            hdT_ps = ps.tile([P, GP], F32, tag="gt")
            nc.tensor.transpose(hdT_ps[:, 0:P], hd[:], identt[:])
            hdT = sb.tile([P, P], F32, tag="hdT")
            nc.scalar.copy(hdT[:], hdT_ps[:, 0:P])
            qps = ps.tile([P, GP], F32, tag="kr")
            nc.tensor.matmul(qps[:, 0:P], lhsT=hdT[:], rhs=wqTt[:],
                             start=True, stop=True)
            nc.scalar.copy(qsb[:, b * P:(b + 1) * P], qps[:, 0:P])

        # ---------------- main: edge groups + per-block flush ----------------
        ngr = S // GROUP
        for b in range(nblk):
            srct = sb.tile([P, S], I32, tag="srct")
            nc.sync.dma_start(srct[:], srcs[b, :, :])
            dcolt = sb.tile([P, S], F32, tag="dcolt")
            nc.sync.dma_start(dcolt[:], dcol[b, :, :])
            drowt = sb.tile([1, S * P], BF16, tag="drowt")
            nc.sync.dma_start(drowt[:], drow[b:b + 1, :])

            acc = psa.tile([P, D + H], F32, tag="acc")
            q_blk = qsb[:, b * P:(b + 1) * P]

            for g in range(ngr):
                # gather 512 h_src rows (HW contract: one index per partition)
                G4 = gp.tile([P, GP], F32, tag="g4")
                for j in range(GROUP):
                    s = g * GROUP + j
                    nc.gpsimd.indirect_dma_start(
                        out=G4[:, j * P:(j + 1) * P],
                        out_offset=None,
                        in_=hsrc[:, :],
                        in_offset=IndirectOffsetOnAxis(
                            ap=srct[:, s:s + 1], axis=0),
                    )
                # Gt4[in, x] = G4[x, in]
                Gt_ps = ps.tile([P, GP], F32, tag="gt")
                for j in range(GROUP):
                    nc.tensor.transpose(
                        Gt_ps[:, j * P:(j + 1) * P], G4[:, j * P:(j + 1) * P],
                        identt[:])
                Gt = sb.tile([P, GP], F32, tag="gtsb")
                nc.scalar.copy(Gt[:], Gt_ps[:])
                # drel broadcast down partitions: drow_bc[d, x] = drel[x]
                drps = ps.tile([P, GP], F32, tag="drow")
                nc.tensor.matmul(
                    drps[:], lhsT=onesbt[:],
                    rhs=drowt[:, g * GP:(g + 1) * GP],
                    start=True, stop=True)
                # one-hots: P4[e, (j,d)] / PT4[d, x]
                P4 = sb.tile([P, GP], F32, tag="p4")
                nc.vector.tensor_tensor(
                    out=P4[:].rearrange("p (a b) -> p a b", a=GROUP),
                    in0=dcolt[:, g * GROUP:(g + 1) * GROUP]
                        .unsqueeze(2).to_broadcast([P, GROUP, P]),
                    in1=iotart[:].unsqueeze(1).to_broadcast([P, GROUP, P]),
                    op=mybir.AluOpType.is_equal)
                PT4 = sb.tile([P, GP], F32, tag="pt4")
                nc.vector.tensor_tensor(
                    out=PT4[:],
                    in0=iotact[:].to_broadcast([P, GP]),
                    in1=drps[:],
                    op=mybir.AluOpType.is_equal)
                # krT4[od, x] = k[src_x, od] ; qeT4[od, x] = q[drel_x, od]
                krps = ps.tile([P, GP], F32, tag="kr")
                nc.tensor.matmul(krps[:], lhsT=mmv(wkTt[:]), rhs=mmv(Gt[:]),
                                 start=True, stop=True)
                qeps = ps.tile([P, GP], F32, tag="qe")
                nc.tensor.matmul(qeps[:], lhsT=mmv(q_blk), rhs=mmv(PT4[:]),
                                 start=True, stop=True)
                qesb = sb.tile([P, GP], F32, tag="qesb")
                nc.scalar.copy(qesb[:], qeps[:])
                prod = sb.tile([P, GP], F32, tag="prod")
                nc.vector.tensor_tensor(out=prod[:], in0=krps[:], in1=qesb[:],
                                        op=mybir.AluOpType.mult)
                # t_row[x, h] = sum_{od in h} prod[od, x]  (4 tiny matmuls)
                trow = ps.tile([P, GH], F32, tag="trow")
                for j in range(GROUP):
                    nc.tensor.matmul(trow[:, j * H:(j + 1) * H],
                                     lhsT=prod[:, j * P:(j + 1) * P],
                                     rhs=hmaskt[:], start=True, stop=True)
                # cat[e, j, 128:136] = exp(scale*t) ; cat[e, j, 0:128] = e * G4
                cat = sb.tile([P, GC], F32, tag="cat")
                catv = cat[:].rearrange("p (a f) -> p a f", a=GROUP)
                nc.scalar.activation(catv[:, :, D:D + H],
                                     trow[:].rearrange("p (a h) -> p a h",
                                                       a=GROUP),
                                     mybir.ActivationFunctionType.Exp,
                                     scale=SCALE)
                nc.vector.tensor_tensor(
                    out=catv[:, :, 0:D].rearrange("p a (h k) -> p a h k", h=H),
                    in0=G4[:].rearrange("p (a h k) -> p a h k", a=GROUP, h=H),
                    in1=catv[:, :, D:D + H]
                        .unsqueeze(3).to_broadcast([P, GROUP, H, DK]),
                    op=mybir.AluOpType.mult)
                # scatter: acc[d, :] += P4_j.T @ [msg | e]
                for j in range(GROUP):
                    nc.tensor.matmul(
                        acc[:],
                        lhsT=P4[:, j * P:(j + 1) * P],
                        rhs=catv[:, j, :],
                        start=(g == 0 and j == 0),
                        stop=(g == ngr - 1 and j == GROUP - 1))

                if debug_dump and b == 0 and g == 0:
                    for nm, t, w in (("g4", G4, GP), ("p4", P4, GP),
                                     ("pt4", PT4, GP), ("gt", Gt, GP),
                                     ("qesb", qesb, GP), ("prod", prod, GP),
                                     ("cat", cat, GC)):
                        nc.sync.dma_start(dbg[nm][:, 0:w], t[:])
                    for nm, t, w in (("drps", drps, GP), ("krps", krps, GP),
                                     ("trow", trow, GH)):
                        tmp = sb.tile([P, w], F32, tag="dbgtmp")
                        nc.scalar.copy(tmp[:], t[:])
                        nc.sync.dma_start(dbg[nm][:, 0:w], tmp[:])

            # ---------------- flush block b ----------------
            hd2 = sb.tile([P, D], F32, tag="hd")
            nc.sync.dma_start(hd2[:], hdst[b * P:(b + 1) * P, :])
            hdT_ps2 = ps.tile([P, GP], F32, tag="gt")
            nc.tensor.transpose(hdT_ps2[:, 0:P], hd2[:], identt[:])
            hdT2 = sb.tile([P, P], F32, tag="hdT")
            nc.scalar.copy(hdT2[:], hdT_ps2[:, 0:P])
            dkps = ps.tile([P, GP], F32, tag="kr")
            nc.tensor.matmul(dkps[:, 0:P], lhsT=hdT2[:], rhs=wkTt[:],
                             start=True, stop=True)
            sprod = sb.tile([P, D], F32, tag="sprod")
            nc.vector.tensor_tensor(out=sprod[:], in0=dkps[:, 0:P], in1=q_blk,
                                    op=mybir.AluOpType.mult)
            st = sb.tile([P, H], F32, tag="st")
            nc.vector.reduce_sum(
                out=st[:], in_=sprod[:].rearrange("p (h k) -> p h k", h=H),
                axis=mybir.AxisListType.X)
            se = sb.tile([P, H], F32, tag="se")
            nc.scalar.activation(se[:], st[:],
                                 mybir.ActivationFunctionType.Exp, scale=SCALE)
            den = sb.tile([P, H], F32, tag="den")
            nc.vector.tensor_add(out=den[:], in0=acc[:, D:D + H], in1=se[:])
            rec = sb.tile([P, H], F32, tag="rec")
            nc.vector.reciprocal(rec[:], den[:])
            o1 = sb.tile([P, D], F32, tag="o1")
            nc.vector.tensor_tensor(
                out=o1[:].rearrange("p (h k) -> p h k", h=H),
                in0=hd2[:].rearrange("p (h k) -> p h k", h=H),
                in1=se[:].unsqueeze(2).to_broadcast([P, H, DK]),
                op=mybir.AluOpType.mult)
            o2 = sb.tile([P, D], F32, tag="o2")
            nc.vector.tensor_add(out=o2[:], in0=o1[:], in1=acc[:, 0:D])
            o3 = sb.tile([P, D], F32, tag="o3")
            nc.vector.tensor_tensor(
                out=o3[:].rearrange("p (h k) -> p h k", h=H),
                in0=o2[:].rearrange("p (h k) -> p h k", h=H),
                in1=rec[:].unsqueeze(2).to_broadcast([P, H, DK]),
                op=mybir.AluOpType.mult)
            nc.sync.dma_start(out[b * P:(b + 1) * P, :], o3[:])

    nc.compile()
    return nc


# --------------------------------------------------------------------------
# host-side sharding / slotting
# --------------------------------------------------------------------------

def preprocess(h_src, h_dst, Wk, Wq, src_idx, dst_idx, ncores=NCORES):
    """Bucket edges by dst range, build per-core slot arrays."""
    n_src = h_src.shape[0]
    n_dst = h_dst.shape[0]
    dpc = n_dst // ncores                      # dst rows per core
    nblk = (dpc + P - 1) // P                  # 128-dst blocks per core
    ndp = nblk * P

    order = np.argsort(dst_idx, kind="stable")
    dsorted = dst_idx[order].astype(np.int64)
    ssorted = src_idx[order].astype(np.int32)

    # block boundaries for every (core, block)
    starts = np.minimum(
        np.arange(ncores)[:, None] * dpc + np.arange(nblk + 1)[None, :] * P,
        (np.arange(ncores)[:, None] + 1) * dpc)          # [ncores, nblk+1]
    cuts = np.searchsorted(dsorted, starts.ravel()).reshape(ncores, nblk + 1)
    counts = np.diff(cuts, axis=1)                       # [ncores, nblk]
    max_n = int(counts.max())
    S = -(-max_n // P)          # sub-chunks needed
    S = -(-S // GROUP) * GROUP  # round to GROUP multiple

    srcs = np.zeros((ncores, nblk, P, S), np.int32)
    dcol = np.full((ncores, nblk, P, S), PAD_DREL, np.float32)
    drow = np.full((ncores, nblk, S * P), PAD_DREL, np.float32)
    for c in range(ncores):
        for b in range(nblk):
            lo, hi = cuts[c, b], cuts[c, b + 1]
            n = hi - lo
            if n == 0:
                continue
            ss = ssorted[lo:hi]
            dd = (dsorted[lo:hi] - starts[c, b]).astype(np.float32)
            i = np.arange(n)
            s_i, p_i = i // P, i % P
            srcs[c, b, p_i, s_i] = ss
            dcol[c, b, p_i, s_i] = dd
            drow[c, b, s_i * P + p_i] = dd
    drow = drow.astype(ml_dtypes.bfloat16)

    iotar = np.tile(np.arange(P, dtype=np.float32), (P, 1))
    iotac = np.arange(P, dtype=np.float32)[:, None].copy()
    hmaskv = (np.arange(P)[:, None] // DK ==
              np.arange(H)[None, :]).astype(np.float32)
    ident = np.eye(P, dtype=np.float32)
    onesb = np.ones((1, P), ml_dtypes.bfloat16)
    wkT = np.ascontiguousarray(Wk.T).astype(np.float32)
    wqT = np.ascontiguousarray(Wq.T).astype(np.float32)

    hs = np.ascontiguousarray(h_src, np.float32)
    in_maps = []
    for c in range(ncores):
        hd = np.zeros((ndp, D), np.float32)
        hd[:dpc] = h_dst[c * dpc:(c + 1) * dpc]
        in_maps.append({
            "hsrc": hs, "hdst": hd, "wkT": wkT, "wqT": wqT,
            "srcs": srcs[c], "dcol": dcol[c], "drow": drow[c],
            "iotar": iotar, "iotac": iotac, "hmask": hmaskv,
            "ident": ident, "onesb": onesb,
        })
    return in_maps, dict(n_src=n_src, nblk=nblk, S=S, dpc=dpc, ndp=ndp)


# --------------------------------------------------------------------------
# PJRT runner with persistent jit (axon path)
# --------------------------------------------------------------------------

class Runner:
    def __init__(self, nc, n_cores=NCORES):
        import jax
        from jax.experimental.shard_map import shard_map
        from jax.sharding import Mesh, PartitionSpec

        from concourse import bass2jax
        bass2jax.install_neuronx_cc_hook()

        partition_name = (nc.partition_id_tensor.name
                          if nc.partition_id_tensor else None)
        in_names, out_names, out_avals = [], [], []
        for alloc in nc.m.functions[0].allocations:
            if not isinstance(alloc, mybir.MemoryLocationSet):
                continue
            name = alloc.memorylocations[0].name
            if alloc.kind == "ExternalInput":
                if name != partition_name:
                    in_names.append(name)
            elif alloc.kind == "ExternalOutput":
                out_names.append(name)
                out_avals.append(jax.core.ShapedArray(
                    tuple(alloc.tensor_shape), mybir.dt.np(alloc.dtype)))
        self.in_names, self.out_names, self.out_avals = \
            in_names, out_names, out_avals
        self.n_cores = n_cores
        all_names = tuple(in_names + out_names)
        if partition_name is not None:
            all_names = all_names + (partition_name,)
        n_params = len(in_names)
        donate = tuple(range(n_params, n_params + len(out_names)))

        def _body(*args):
            operands = list(args)
            if partition_name is not None:
                operands.append(bass2jax.partition_id_tensor())
            outs = bass2jax._bass_exec_p.bind(
                *operands,
                out_avals=tuple(out_avals),
                in_names=all_names,
                out_names=tuple(out_names),
                lowering_input_output_aliases=(),
                sim_require_finite=True,
                sim_require_nnan=True,
                nc=nc,
            )
            return tuple(outs)

        devices = jax.devices()[:n_cores]
        mesh = Mesh(np.asarray(devices), ("core",))
        nin = n_params + len(out_names)
        self._fn = jax.jit(
            shard_map(_body, mesh=mesh,
                      in_specs=(PartitionSpec("core"),) * nin,
                      out_specs=(PartitionSpec("core"),) * len(out_names),
                      check_rep=False),
            donate_argnums=donate, keep_unused=True)
        self._sharding = jax.NamedSharding(mesh, PartitionSpec("core"))

    def concat_inputs(self, in_maps):
        return [
            np.concatenate([np.asarray(m[name]) for m in in_maps], axis=0)
            for name in self.in_names
        ]

    def place(self, concat_in):
        """Ship inputs to the devices once; reusable across calls."""
        import jax
        return [jax.device_put(a, self._sharding) for a in concat_in]

    def make_zeros(self):
        """Fresh device-resident zero output buffers (donated per call)."""
        import jax
        return [
            jax.device_put(
                np.zeros((self.n_cores * a.shape[0], *a.shape[1:]), a.dtype),
                self._sharding)
            for a in self.out_avals
        ]

    def run(self, placed_in, zeros):
        return self._fn(*placed_in, *zeros)

    def __call__(self, concat_in):
        outs = self._fn(*concat_in, *self.make_zeros())
        return [np.asarray(o) for o in outs]


_CACHE = {}


def _get_runner(n_src, nblk, S):
    key = (n_src, nblk, S)
    if key not in _CACHE:
        nc = build_program(n_src, nblk, S)
        _CACHE[key] = Runner(nc)
    return _CACHE[key]


def kernel(h_src, h_dst, Wk, Wq, src_idx, dst_idx):
    h_src = np.asarray(h_src, np.float32)
    h_dst = np.asarray(h_dst, np.float32)
    Wk = np.asarray(Wk, np.float32)
    Wq = np.asarray(Wq, np.float32)
    src_idx = np.asarray(src_idx, np.int32)
    dst_idx = np.asarray(dst_idx, np.int32)

    in_maps, meta = preprocess(h_src, h_dst, Wk, Wq, src_idx, dst_idx)
    runner = _get_runner(meta["n_src"], meta["nblk"], meta["S"])
    concat_in = runner.concat_inputs(in_maps)
    outs = runner(concat_in)
    full = outs[runner.out_names.index("out")]
    full = full.reshape(NCORES, meta["ndp"], D)
    return np.ascontiguousarray(
        full[:, :meta["dpc"], :].reshape(-1, D))
